# revision 29
# baseline (speedup 1.0000x reference)
"""GroupedQueryAttention (B=1, S=4096, D=1024, G=16 heads, DH=64) on 8 TRN2 NeuronCores.

Sharding: tensor-parallel over heads. Core c computes heads {2c, 2c+1}:
  - Q/K/V projections with column-sliced weights (128 out-dims per core),
    producing Q^T/K^T in [dout, seq] layout (host pre-transposes inputs).
  - V^T is built per head padded to 80 rows with an all-ones row 64; SBUF->
    SBUF xbar DMA transposes turn it into V-natural [kpos, 80] chunks whose
    col 64 is the ones column that makes the PV matmul accumulate the
    softmax denominator for free.
  - Flash-style attention without max-subtraction (scores are tiny:
    |s/8| < ~3), exp on ScalarE with fused 1/8 scale + per-key mask bias.
  - Output projection with row-sliced Wo produces a partial (4096, 1024)
    bf16 output per core; host sums the 8 partials and adds bo.

All matmul operands are bf16 (fp32 PSUM accumulation). K/V inputs stream
through quarter-chunk tiles sized so DMAs stay ahead of the projection
matmuls; Q is loaded and projected per quarter inside the attention phase
so its HBM traffic overlaps compute. Attention runs in (quarter=1024 q,
head) groups whose PV accumulators alternate between two PSUM bank pairs
so the PE stream never gaps at group boundaries (keeps the HAM clock at
K=8/8).
"""

import os
import sys

for _p in ("/opt/trn_rl_repo", "/root/.axon_site/_ro/trn_rl_repo"):
    if os.path.isdir(_p) and _p not in sys.path:
        sys.path.insert(0, _p)

from contextlib import ExitStack

import ml_dtypes
import numpy as np

import concourse.bass as bass
import concourse.mybir as mybir
import concourse.tile as tile
from concourse import bacc
from concourse.bass_utils import run_bass_kernel_spmd

S = 4096          # sequence length
D = 1024          # model dim
G = 16            # heads
DH = 64           # head dim
P = 128           # partitions
QT = 512          # q-tile (moving free dim)
KC = 128          # k-chunk
NCORES = 8
HPC = G // NCORES             # heads per core = 2
N_ST = S // QT                # 8 s-tiles of 512
N_KCH = D // P                # 8 contraction chunks for projections
N_KC = S // KC                # 32 k-chunks for attention
DSL = P                       # per-core dout slice (2 heads * 64)
QPH = 2                       # q-tiles per attention group (quarter)
N_GRP = N_ST // QPH           # 4 quarters
QC = S // N_GRP               # 1024 columns per quarter chunk

F32 = mybir.dt.float32
BF16 = mybir.dt.bfloat16
BF = ml_dtypes.bfloat16

_CACHE = {}


def _build_nc():
    key = "nc"
    if key in _CACHE:
        return _CACHE[key]

    nc = bacc.Bacc(
        "TRN2", target_bir_lowering=False, debug=False, num_devices=NCORES
    )

    xqT = nc.dram_tensor("xqT", [D, S], BF16, kind="ExternalInput").ap()
    xkT = nc.dram_tensor("xkT", [D, S], BF16, kind="ExternalInput").ap()
    xvT = nc.dram_tensor("xvT", [D, S], BF16, kind="ExternalInput").ap()
    wqT = nc.dram_tensor("wqT", [N_KCH, P, DSL], BF16, kind="ExternalInput").ap()
    wkT = nc.dram_tensor("wkT", [N_KCH, P, DSL], BF16, kind="ExternalInput").ap()
    wvT = nc.dram_tensor("wvT", [N_KCH, P, DSL], BF16, kind="ExternalInput").ap()
    woT = nc.dram_tensor("woT", [DSL, D], BF16, kind="ExternalInput").ap()
    bq = nc.dram_tensor("bq", [DSL, 1], F32, kind="ExternalInput").ap()
    bk = nc.dram_tensor("bk", [DSL, 1], F32, kind="ExternalInput").ap()
    bv = nc.dram_tensor("bv", [DSL, 1], F32, kind="ExternalInput").ap()
    mbias = nc.dram_tensor("mbias", [P, N_KC], F32, kind="ExternalInput").ap()
    out_d = nc.dram_tensor("out", [S, D], BF16, kind="ExternalOutput").ap()

    with tile.TileContext(nc) as tc, ExitStack() as ctx:
        consts = ctx.enter_context(tc.tile_pool(name="consts", bufs=1))
        big = ctx.enter_context(tc.tile_pool(name="big", bufs=1))
        et_pool = ctx.enter_context(tc.tile_pool(name="et", bufs=6))
        small = ctx.enter_context(tc.tile_pool(name="small", bufs=2))
        oevict = ctx.enter_context(tc.tile_pool(name="oevict", bufs=4))
        # streamed K/V quarter-chunk tiles: 16 slots per tensor tag = 2
        # quarters of lookahead so chunk DMAs stay ahead of the proj matmuls
        xc = ctx.enter_context(tc.tile_pool(name="xc", bufs=16))
        # streamed Q quarter-chunk tiles, loaded inside the attention phase
        xq = ctx.enter_context(tc.tile_pool(name="xq", bufs=16))
        # PSUM: 'sc' 2 slots x 2 banks + 'pv' 4 slots x 1 bank = 8 banks
        ps_sc = ctx.enter_context(tc.tile_pool(name="ps_sc", bufs=2, space="PSUM"))
        ps_pv = ctx.enter_context(tc.tile_pool(name="ps_pv", bufs=4, space="PSUM"))

        # ---- constants (scalar-queue DMAs, parallel to x loads on sync) ----
        w_s = {}
        for name, wd in (("k", wkT), ("v", wvT), ("q", wqT)):
            w = consts.tile([P, N_KCH * DSL], BF16, tag=f"w{name}")
            for kc in range(N_KCH):
                nc.scalar.dma_start(w[:, kc * DSL:(kc + 1) * DSL], wd[kc])
            w_s[name] = w
        wo_s = consts.tile([DSL, D], BF16, tag="wo")
        nc.scalar.dma_start(wo_s[:], woT)
        b_s = {}
        for name, bd in (("q", bq), ("k", bk), ("v", bv)):
            b = consts.tile([DSL, 1], F32, tag=f"b{name}")
            nc.scalar.dma_start(b[:], bd)
            b_s[name] = b
        mb_s = consts.tile([P, N_KC], F32, tag="mb")
        nc.scalar.dma_start(mb_s[:], mbias)

        def load_quarter(pool, tag, xd, qtr, split=False, eng=None):
            """Load one quarter's 8 contraction chunks; with split=True the
            chunks alternate between the sync HWDGE and gpsimd SWDGE queues
            so the two DMA paths stream in parallel."""
            qsl = slice(qtr * QC, (qtr + 1) * QC)
            tiles = []
            for kc in range(N_KCH):
                t = pool.tile([P, QC], BF16, tag=tag, name=f"{tag}{qtr}{kc}")
                e = eng or (nc.gpsimd if (split and kc % 2) else nc.sync)
                e.dma_start(t[:], xd[kc * P:(kc + 1) * P, qsl])
                tiles.append(t)
            return tiles

        # ---- resident activations ----
        QTs = big.tile([P, S], BF16, tag="QTs")      # Q^T  [dout, s]
        KTs = big.tile([P, S], BF16, tag="KTs")      # K^T  [dout, s]
        # V^T per head, padded to 80 rows: rows 0-63 V data, row 64 ones
        # (becomes the PV ones-column after transpose -> softmax denom),
        # rows 65-79 zero pad (xbar transpose needs partition%16==0).
        VTh = [
            big.tile([DH + 16, S], BF16, tag=f"VTh{h}", name=f"VTh{h}")
            for h in range(HPC)
        ]
        # V natural per head: [k-part, chunk, 80]; col DH is the ones column.
        Vnat = [
            big.tile([P, N_KC, DH + 16], BF16, tag=f"Vn{h}", name=f"Vnat{h}")
            for h in range(HPC)
        ]
        attnT = big.tile([P, S], BF16, tag="attnT")  # normalized attn^T [din, s]

        for h in range(HPC):
            nc.vector.memset(VTh[h][DH:DH + 16, :], 0.0)
            nc.vector.memset(VTh[h][DH:DH + 1, :], 1.0)

        def project(name, dst, st, xtiles, stl):
            """One 512-wide s-tile of the `name` projection into dst (bf16)."""
            sl = slice(st * QT, (st + 1) * QT)
            xsl = slice(stl * QT, (stl + 1) * QT)
            ps = ps_sc.tile([P, QT], F32, tag="sc")
            for kc in range(N_KCH):
                nc.tensor.matmul(
                    ps[:],
                    w_s[name][:, kc * DSL:(kc + 1) * DSL],
                    xtiles[kc][:, xsl],
                    start=(kc == 0),
                    stop=(kc == N_KCH - 1),
                )
            # bias add + bf16 cast on VectorE (keeps ScalarE free for exp)
            nc.vector.tensor_add(
                dst[:, sl], ps[:], b_s[name][:].to_broadcast((P, QT))
            )

        def project_v(st, xtiles, stl):
            """V projection s-tile, split per head into the padded VTh."""
            sl = slice(st * QT, (st + 1) * QT)
            xsl = slice(stl * QT, (stl + 1) * QT)
            ps = ps_sc.tile([P, QT], F32, tag="sc")
            for kc in range(N_KCH):
                nc.tensor.matmul(
                    ps[:],
                    w_s["v"][:, kc * DSL:(kc + 1) * DSL],
                    xtiles[kc][:, xsl],
                    start=(kc == 0),
                    stop=(kc == N_KCH - 1),
                )
            for h in range(HPC):
                hs = slice(h * DH, (h + 1) * DH)
                nc.vector.tensor_add(
                    VTh[h][0:DH, sl], ps[hs, :],
                    b_s["v"][hs, 0:1].to_broadcast((DH, QT)),
                )

        def proj_kv_quarter(qtr, kt, vt):
            for stl in range(QC // QT):
                st = qtr * (QC // QT) + stl
                project("k", KTs, st, kt, stl)
                project_v(st, vt, stl)
                # one xbar transpose per (head, s-tile pair): out chunk j
                # gets k-positions at partition s%128; batched 1024-wide so
                # only 8 transposes sit on the scalar queue ahead of the exps
                if st % 2 == 1:
                    c0 = (st - 1) * (QT // KC)
                    for h in range(HPC):
                        nc.scalar.dma_start(
                            Vnat[h][:, c0:c0 + 2 * (QT // KC), 0:DH + 16],
                            VTh[h][:, (st - 1) * QT:(st + 1) * QT],
                            transpose=True,
                        )

        # ---- phase 1: K,V quarters 0-1 projected up front; quarters 2-3
        # are loaded+projected interleaved into group-0-head-0's kc loop
        # (flash accumulation only needs chunk kc ready when kc runs) ----
        kt0 = load_quarter(xc, "xk", xkT, 0, split=True)
        vt0 = load_quarter(xc, "xv", xvT, 0, split=True)
        qtiles0 = load_quarter(xq, "xq", xqT, 0)
        kt1 = load_quarter(xc, "xk", xkT, 1, split=True)
        vt1 = load_quarter(xc, "xv", xvT, 1, split=True)
        proj_kv_quarter(0, kt0, vt0)
        proj_kv_quarter(1, kt1, vt1)

        # ---- phase 2: Q proj + attention + output projection, per quarter --
        def project_q(qtr, qtiles):
            for stl in range(QC // QT):
                project("q", QTs, qtr * (QC // QT) + stl, qtiles, stl)

        def attn_kc_range(grp, h, pvs, kc_lo, kc_hi):
            q0 = grp * QPH * QT
            hs = slice(h * DH, (h + 1) * DH)
            for kc in range(kc_lo, kc_hi):
                ks = slice(kc * KC, (kc + 1) * KC)
                sc = ps_sc.tile([P, QPH * QT], F32, tag="sc")
                for j in range(QPH):
                    nc.tensor.matmul(
                        sc[:, j * QT:(j + 1) * QT],
                        KTs[hs, ks],
                        QTs[hs, q0 + j * QT:q0 + (j + 1) * QT],
                        start=True, stop=True,
                    )
                et = et_pool.tile([P, QPH * QT], BF16, tag="et")
                nc.scalar.activation(
                    et[:], sc[:],
                    mybir.ActivationFunctionType.Exp,
                    bias=mb_s[:, kc:kc + 1], scale=0.125,
                )
                for j in range(QPH):
                    nc.tensor.matmul(
                        pvs[j][:],
                        Vnat[h][:, kc, 0:DH + 1],
                        et[:, j * QT:(j + 1) * QT],
                        start=(kc == 0), stop=(kc == N_KC - 1),
                    )

        def attn_finish(grp, h, pvs):
            # normalize: attnT[hs, q] = pv[0:DH] * (1/pv[DH])
            q0 = grp * QPH * QT
            hs = slice(h * DH, (h + 1) * DH)
            for j in range(QPH):
                qsl = slice(q0 + j * QT, q0 + (j + 1) * QT)
                den = small.tile([1, QT], F32, tag="den")
                nc.vector.tensor_copy(den[:], pvs[j][DH:DH + 1, :])
                rec = small.tile([1, QT], F32, tag="rec")
                # approx_fast needs an SBUF source (PSUM input misreads)
                nc.vector.reciprocal_approx_fast(rec[:], den[:])
                bc = small.tile([DH, QT], F32, tag="bc")
                nc.gpsimd.partition_broadcast(bc[:], rec[:])
                nc.vector.tensor_mul(attnT[hs, qsl], pvs[j][0:DH, :], bc[:])

        def alloc_pvs(grp, h):
            return [
                ps_pv.tile([DH + 1, QT], F32, tag="pv", name=f"pv{grp}{h}{j}")
                for j in range(QPH)
            ]

        # group 0 head 0: interleave its kc loop with KV quarters 2-3 so
        # their loads and projections hide behind attention compute
        project_q(0, qtiles0)
        pvs00 = alloc_pvs(0, 0)
        attn_kc_range(0, 0, pvs00, 0, 2 * (N_KC // N_GRP))
        kt2 = load_quarter(xc, "xk", xkT, 2, split=True)
        vt2 = load_quarter(xc, "xv", xvT, 2, split=True)
        proj_kv_quarter(2, kt2, vt2)
        attn_kc_range(0, 0, pvs00, 2 * (N_KC // N_GRP), 3 * (N_KC // N_GRP))
        kt3 = load_quarter(xc, "xk", xkT, 3, split=True)
        vt3 = load_quarter(xc, "xv", xvT, 3, split=True)
        proj_kv_quarter(3, kt3, vt3)
        attn_kc_range(0, 0, pvs00, 3 * (N_KC // N_GRP), N_KC)
        attn_finish(0, 0, pvs00)

        for grp in range(N_GRP):
            if grp + 1 < N_GRP:
                qtiles_next = load_quarter(xq, "xq", xqT, grp + 1)
            for h in range(HPC):
                if grp == 0 and h == 0:
                    continue  # done above, interleaved with KV q2-3
                pvs = alloc_pvs(grp, h)
                attn_kc_range(grp, h, pvs, 0, N_KC)
                attn_finish(grp, h, pvs)
            # output projection for this quarter's 8 128-wide s-chunks
            for j in range(QPH * QT // P):
                st = grp * (QPH * QT // P) + j
                for nt in range(D // QT):
                    po = ps_pv.tile([P, QT], F32, tag="pv", name=f"po{grp}{j}{nt}")
                    nc.tensor.matmul(
                        po[:],
                        attnT[:, st * P:(st + 1) * P],
                        wo_s[:, nt * QT:(nt + 1) * QT],
                        start=True, stop=True,
                    )
                    ot = oevict.tile([P, QT], BF16, tag="ot")
                    nc.vector.tensor_copy(ot[:], po[:])
                    # final quarter: scalar queue is exp-free by then, so
                    # split its stores across both HWDGE queues to halve
                    # the output drain tail
                    eng = (
                        nc.scalar if (grp == N_GRP - 1 and nt == 1) else nc.sync
                    )
                    eng.dma_start(
                        out_d[st * P:(st + 1) * P, nt * QT:(nt + 1) * QT], ot[:]
                    )
            if grp + 1 < N_GRP:
                project_q(grp + 1, qtiles_next)

    nc.compile()
    _CACHE[key] = nc
    return nc


def _prep_in_maps(query, key, value, mask, Wq, bq, Wk, bk, Wv, bv, Wo, bo):
    f = np.float32
    qT = np.ascontiguousarray(np.asarray(query, dtype=f)[0].T).astype(BF)
    kT = np.ascontiguousarray(np.asarray(key, dtype=f)[0].T).astype(BF)
    vT = np.ascontiguousarray(np.asarray(value, dtype=f)[0].T).astype(BF)
    mb = np.where(np.asarray(mask)[0] == 0, f(-1e9), f(0.0)).astype(f)
    mb = np.ascontiguousarray(mb.reshape(N_KC, KC).T)  # [128, 32]
    WqT, WkT, WvT, WoT = (
        np.ascontiguousarray(np.asarray(W, dtype=f).T).astype(BF)
        for W in (Wq, Wk, Wv, Wo)
    )
    in_maps = []
    for c in range(NCORES):
        cs = slice(c * DSL, (c + 1) * DSL)
        in_maps.append({
            "xqT": qT, "xkT": kT, "xvT": vT,
            "wqT": np.ascontiguousarray(WqT[:, cs]).reshape(N_KCH, P, DSL),
            "wkT": np.ascontiguousarray(WkT[:, cs]).reshape(N_KCH, P, DSL),
            "wvT": np.ascontiguousarray(WvT[:, cs]).reshape(N_KCH, P, DSL),
            "woT": np.ascontiguousarray(WoT[cs, :]),
            "bq": np.ascontiguousarray(bq[cs].astype(f, copy=False)).reshape(DSL, 1),
            "bk": np.ascontiguousarray(bk[cs].astype(f, copy=False)).reshape(DSL, 1),
            "bv": np.ascontiguousarray(bv[cs].astype(f, copy=False)).reshape(DSL, 1),
            "mbias": mb,
        })
    return in_maps


def run(inputs, trace=False, trace_kwargs=None):
    nc = _build_nc()
    in_maps = _prep_in_maps(**inputs)
    res = run_bass_kernel_spmd(
        nc, in_maps, core_ids=list(range(NCORES)), trace=trace,
        **(trace_kwargs or {}),
    )
    bo = np.asarray(inputs["bo"], dtype=np.float32)
    acc = np.zeros((S, D), dtype=np.float32)
    for r in res.results:
        acc += np.asarray(r["out"], dtype=np.float32)
    out = (acc + bo[None, :]).astype(np.float32)[None]
    return out, res


def kernel(**inputs):
    out, _ = run(inputs, trace=False)
    return out
